# revision 1
# baseline (speedup 1.0000x reference)
"""Continual-attention Trainium2 kernel (8 NeuronCores, SPMD).

Sharding: core c -> batch b = c//2, head-group g = c%2 (4 heads each).
Per (b,h) computes S^T[k,q] = K Q^T via PE with K=64 contraction row-tiled
onto alternating halves of the PE array (2x concurrency), exp on ScalarE
over groups of 3 k-tiles (1536-col PSUM spans), multiplicative 0/1 masks
on DVE, then O^T[65,q] (64 dims + denominator row via ones column in V)
accumulated on PE. Normalization + final transpose happen on host.
"""

import sys

sys.path.insert(0, "/opt/trn_rl_repo")

import numpy as np

B, L, H, D = 4, 2048, 8, 64
TRAIN = 1536
TEST = L - TRAIN            # 512
NCH = 64                    # test chunks
CH = TEST // NCH            # 8
HPC = 4                     # heads per core
NCORES = 8
KT = L // 128               # 16 k-tiles

KQV = 2 * L + KT * 65       # combined per-head columns: kt | qt | vw
QOFF = L                    # qt column offset in kqv
VOFF = 2 * L                # vw column offset in kqv

GROUP = 3                   # k-tiles per PSUM tile / EXP instruction
SPW = 512 * GROUP           # PSUM tile width
PIPE = 2                    # groups of AV matmuls held back

LAST_RESULT = None          # BassKernelResults of the most recent run
_PROG = None                # cached compiled Bass program


def _split_multi_waits(nc, mybir):
    """This container's walrus accepts at most one semaphore wait per
    instruction; Tile's tail drains can carry several. Hoist extras onto
    NoOps inserted immediately before, on the same engine."""
    for f in nc.m.functions:
        for bb in f.blocks:
            insts = list(bb.instructions)
            out = []
            changed = False
            for inst in insts:
                si = inst.sync_info
                if si is not None and len(si.on_wait) > 1:
                    waits = list(si.on_wait)
                    for w in waits[:-1]:
                        nop = mybir.InstNoOp(
                            name=f"waitnop-{nc.next_id()}", ins=[], outs=[]
                        )
                        nop.engine = inst.engine
                        nop.sync_info = mybir.SyncInfo(on_wait=[w], on_update=[])
                        out.append(nop)
                    inst.sync_info = mybir.SyncInfo(
                        on_wait=[waits[-1]], on_update=list(si.on_update)
                    )
                    changed = True
                out.append(inst)
            if changed:
                bb.instructions = out


def _geom(kp, gq):
    """(off, w) of k-tile kp's q-span inside q-group gq (512 wide)."""
    if kp <= 11:
        off = max(0, 128 * kp - 512 * gq)
        return off, 512 - off
    off = 128 * (kp - 12)
    return off, 128


def _groups(gq):
    """Pack this gq's k-tiles into groups of <=GROUP with bank-aligned
    positions (no matmul output crosses a 512-col PSUM bank boundary).
    Returns list of [(kp, pos, off, w), ...] per group."""
    kps = list(range(4 * (gq + 1))) if gq < 3 else list(range(16))
    out = []
    cur = []
    pos = 0
    for kp in kps:
        off, w = _geom(kp, gq)
        # bank-align: place within current 512 bank if it fits
        bank_rem = -pos % 512
        if bank_rem and w > bank_rem:
            pos += bank_rem
        if len(cur) == GROUP or pos + w > SPW:
            out.append(cur)
            cur = []
            pos = 0
        cur.append((kp, pos, off, w))
        pos += w
    if cur:
        out.append(cur)
    return out


def _build_program():
    import os
    import concourse.bass as bass
    import concourse.mybir as mybir
    import concourse.tile as tile

    rowtile = bool(int(os.environ.get("K_ROWTILE", "0")))
    scalar_dma = bool(int(os.environ.get("K_SCALAR_DMA", "1")))

    f32 = mybir.dt.float32
    fp16 = mybir.dt.float16
    Exp = mybir.ActivationFunctionType.Exp

    nc = bass.Bass()

    kqv_d = nc.dram_tensor("kqv", [HPC, 128, KQV], fp16, kind="ExternalInput")
    mtt_d = nc.dram_tensor("mtt", [128, 12 * 512], fp16, kind="ExternalInput")
    msk_d = nc.dram_tensor("msk", [128, 256], fp16, kind="ExternalInput")
    ot_d = nc.dram_tensor("ot", [HPC, 65, L], fp16, kind="ExternalOutput")

    with tile.TileContext(nc) as tc:
        with (
            tc.tile_pool(name="consts", bufs=1) as consts,
            tc.tile_pool(name="heads", bufs=4) as heads,
            tc.tile_pool(name="ptp", bufs=4) as ptp,
            tc.tile_pool(name="osbp", bufs=3) as osbp,
            tc.tile_pool(name="spp", bufs=2, space="PSUM") as spp,
            tc.tile_pool(name="avp", bufs=2, space="PSUM") as avp,
        ):
            # ---- PE clock warm-up -----------------------------------------
            # The PE HAM clock gate keeps the array at 1.2 GHz until it has
            # seen ~3.4us of sustained matmul activity. Real data only lands
            # at ~11us, so without this the whole first q-group runs at half
            # clock. Dummy matmuls on a zeroed tile from t~0.5us keep the
            # array warm (and inside the MID re-throttle window) until then.
            warm_sb = consts.tile([128, 128], fp16, name="warm_sb")
            nc.gpsimd.memset(warm_sb, 0.0)
            warm_ps = spp.tile([128, SPW], f32, tag="sp", name="warm_ps")
            for _ in range(60):
                nc.tensor.matmul(
                    warm_ps[:, 0:128],
                    lhsT=warm_sb,
                    rhs=warm_sb,
                    start=True,
                    stop=True,
                    skip_group_check=True,
                )

            # ---- input DMAs, all issued up front --------------------------
            msk_sb = consts.tile([128, 256], fp16)
            mtt_sb = consts.tile([128, 12 * 512], fp16)
            kqv_sbs = []
            for h in range(HPC):
                kqv_sbs.append(
                    heads.tile([128, KQV], fp16, tag="kqv", name=f"kqv{h}")
                )

            # Inputs via SWDGE (gpsimd): each dma_start is spread across all
            # 16 SDMA engines, vs HWDGE which serializes one queue per DMA.
            # head 0 split so first S-matmul columns land early.
            eng2 = nc.scalar if scalar_dma else nc.gpsimd
            nc.gpsimd.dma_start(
                out=kqv_sbs[0][:, 0:1024], in_=kqv_d.ap()[0][:, 0:1024]
            )
            nc.gpsimd.dma_start(
                out=kqv_sbs[0][:, QOFF : QOFF + 1024],
                in_=kqv_d.ap()[0][:, QOFF : QOFF + 1024],
            )
            eng2.dma_start(out=msk_sb, in_=msk_d.ap())
            nc.gpsimd.dma_start(
                out=kqv_sbs[0][:, 1024:2048], in_=kqv_d.ap()[0][:, 1024:2048]
            )
            nc.gpsimd.dma_start(
                out=kqv_sbs[0][:, QOFF + 1024 : VOFF],
                in_=kqv_d.ap()[0][:, QOFF + 1024 : VOFF],
            )
            nc.gpsimd.dma_start(
                out=kqv_sbs[0][:, VOFF:KQV], in_=kqv_d.ap()[0][:, VOFF:KQV]
            )
            nc.gpsimd.dma_start(out=mtt_sb, in_=mtt_d.ap())
            nc.gpsimd.dma_start(out=kqv_sbs[1], in_=kqv_d.ap()[1])
            nc.gpsimd.dma_start(out=kqv_sbs[2], in_=kqv_d.ap()[2])
            nc.gpsimd.dma_start(out=kqv_sbs[3], in_=kqv_d.ap()[3])

            mdiag = msk_sb[:, 0:128]
            mchunk = msk_sb[:, 128:256]

            s_idx = 0  # global S-matmul counter for PE row-half alternation
            osb_i = 0
            pending = []  # (av, kqv_sb, grp, pt, last_kp, fin) fin=(h,gq)|None

            def pop_one():
                nonlocal osb_i
                av_, kqv_, grp_, pt_, last_, fin_ = pending.pop(0)
                for kp, pos, off, w in grp_:
                    nc.tensor.matmul(
                        av_[:65, off : off + w],
                        lhsT=kqv_[:, VOFF + 65 * kp : VOFF + 65 * kp + 65],
                        rhs=pt_[:, pos : pos + w],
                        start=kp == 0,
                        stop=kp == last_,
                        skip_group_check=True,
                    )
                if fin_ is not None:
                    h_, gq_ = fin_
                    osb = osbp.tile([65, 512], fp16, name=f"osb{osb_i}")
                    osb_i += 1
                    nc.vector.tensor_copy(osb, av_[:65, :])
                    nc.gpsimd.dma_start(
                        out=ot_d.ap()[h_][:, 512 * gq_ : 512 * gq_ + 512],
                        in_=osb,
                    )

            for h in range(HPC):
                kqv_sb = kqv_sbs[h]
                for gq in range(4):
                    av = avp.tile([128, 512], f32, tag="av")
                    groups = _groups(gq)
                    last_kp = groups[-1][-1][0]

                    for gi, grp in enumerate(groups):
                        span = grp[-1][1] + grp[-1][3]
                        sp = spp.tile([128, SPW], f32, tag="sp")
                        for kp, pos, off, w in grp:
                            if rowtile:
                                half = (
                                    slice(0, 64) if s_idx % 2 == 0 else slice(64, 128)
                                )
                            else:
                                # K/Q rows are duplicated, so a full-128
                                # contraction computes 2*S; EXP scale halves.
                                half = slice(0, 128)
                            s_idx += 1
                            qs = QOFF + 512 * gq + off
                            nc.tensor.matmul(
                                sp[:, pos : pos + w],
                                lhsT=kqv_sb[half, 128 * kp : 128 * kp + 128],
                                rhs=kqv_sb[half, qs : qs + w],
                                start=True,
                                stop=True,
                                skip_group_check=True,
                            )
                        pt = ptp.tile([128, SPW], fp16, tag="pt")
                        nc.scalar.activation(
                            pt[:, 0:span],
                            sp[:, 0:span],
                            Exp,
                            scale=0.125 if rowtile else 0.0625,
                        )
                        if gq == 3 and grp[0][0] <= 11:
                            # per-batch test-train 0/1 mask on DVE; group spans
                            # mtt cols [512*kp0, 512*kp0 + span)
                            m0 = 512 * grp[0][0]
                            nc.vector.tensor_mul(
                                pt[:, 0:span],
                                pt[:, 0:span],
                                mtt_sb[:, m0 : m0 + span],
                            )
                        for kp, pos, off, w in grp:
                            if kp <= 11 and 128 * kp >= 512 * gq:
                                nc.vector.tensor_mul(
                                    pt[:, pos : pos + 128],
                                    pt[:, pos : pos + 128],
                                    mdiag,
                                )
                            elif kp >= 12:
                                nc.vector.tensor_mul(
                                    pt[:, pos : pos + 128],
                                    pt[:, pos : pos + 128],
                                    mchunk,
                                )

                        fin = (h, gq) if gi == len(groups) - 1 else None
                        pending.append((av, kqv_sb, grp, pt, last_kp, fin))
                        while len(pending) > PIPE:
                            pop_one()

            while pending:
                pop_one()

    import concourse.mybir as mybir_mod

    _split_multi_waits(nc, mybir_mod)
    return nc


def _host_inputs(queries, keys, values, attach):
    """Build per-core input maps (host-side layout prep)."""
    f16 = np.float16
    p = np.arange(128)
    f = np.arange(128)
    mdiag = np.where(f[None, :] >= p[:, None], 1.0, 0.0).astype(np.float32)
    mchunk = np.where(
        (p[:, None] // CH == f[None, :] // CH) & (p[:, None] <= f[None, :]),
        1.0,
        0.0,
    ).astype(np.float32)
    msk = np.concatenate([mdiag, mchunk], axis=1)  # [128, 256]

    in_maps = []
    for c in range(NCORES):
        b, g = divmod(c, 2)
        hs = slice(HPC * g, HPC * (g + 1))
        q = queries[b][:, hs, :]          # [L, 4, D]
        k = keys[b][:, hs, :]
        v = values[b][:, hs, :]
        qt = q.transpose(1, 2, 0)         # [4, 64, L]
        kt = k.transpose(1, 2, 0)
        vw = np.empty((HPC, L, 65), np.float32)
        vw[:, :, :64] = v.transpose(1, 0, 2)
        vw[:, :, 64] = 1.0
        # [4, L, 65] -> [4, 128, KT*65] with row p holding tile-chunks
        vw = np.ascontiguousarray(
            vw.reshape(HPC, KT, 128, 65).transpose(0, 2, 1, 3).reshape(HPC, 128, KT * 65)
        )
        # combined [4, 128, KQV]: kt | qt | vw, with K/Q duplicated into
        # partitions 64-127 for PE row-tiling
        kqv = np.empty((HPC, 128, KQV), np.float32)
        kqv[:, :64, 0:L] = kt
        kqv[:, 64:, 0:L] = kt
        kqv[:, :64, QOFF:VOFF] = qt
        kqv[:, 64:, QOFF:VOFF] = qt
        kqv[:, :, VOFF:] = vw
        kg = (np.arange(12)[:, None] * 128 + np.arange(128)[None, :])  # [12,128]
        thr = attach[b][np.arange(TEST) // CH]                          # [512]
        mtt = np.where(kg[:, :, None] <= thr[None, None, :], 1.0, 0.0)  # [12,128,512]
        mtt = np.ascontiguousarray(mtt.transpose(1, 0, 2).reshape(128, 12 * 512))
        in_maps.append(
            {
                "kqv": kqv.astype(f16),
                "mtt": mtt.astype(f16),
                "msk": msk.astype(f16),
            }
        )
    return in_maps


def kernel(queries, keys, values, attach_test_after, train_len):
    global LAST_RESULT, _PROG
    import os

    queries = np.asarray(queries, dtype=np.float32)
    keys = np.asarray(keys, dtype=np.float32)
    values = np.asarray(values, dtype=np.float32)
    attach = np.asarray(attach_test_after).astype(np.int64)
    tl = int(np.asarray(train_len))
    assert queries.shape == (B, L, H, D), queries.shape
    assert tl == TRAIN and attach.shape == (B, NCH)

    from concourse.bass_utils import run_bass_kernel_spmd

    if _PROG is None:
        _PROG = _build_program()

    in_maps = _host_inputs(queries, keys, values, attach)
    trace = bool(int(os.environ.get("KERNEL_TRACE", "0")))
    res = run_bass_kernel_spmd(
        _PROG, in_maps, core_ids=list(range(NCORES)), trace=trace
    )
    LAST_RESULT = res

    out = np.empty((B, L, H * D), np.float32)
    for c in range(NCORES):
        b, g = divmod(c, 2)
        ot = res.results[c]["ot"].astype(np.float32)  # [4, 65, L] (fp16 on dev)
        o = ot[:, :64, :] / ot[:, 64:65, :]           # [4, 64, L]
        out[b, :, 256 * g : 256 * (g + 1)] = (
            o.transpose(2, 0, 1).reshape(L, HPC * D)
        )
    return out



# revision 2
# speedup vs baseline: 1.0419x; 1.0419x over previous
"""Continual-attention Trainium2 kernel v3 (8 NeuronCores, SPMD).

Sharding: core c -> batch b = c//2, head-group g = c%2 (4 heads each).

Per (b,h): S^T[k,q] = K Q^T on PE with 64 data contraction rows (scaled by
sqrt(A), A = 1024*log2(e)/8) plus 64 mask rows folded into the same matmul:
Q rows 64-127 hold per-chunk indicators for test queries, K rows 64-127 hold
-60000*[k > attach[b,n]] steps, so masked logits go hugely negative in PSUM.
The exp+PSUM-drain pass is split between ScalarE (true exp via activation,
scale=ln2/1024) and DVE (Schraudolph: round(A*S + B) as uint16 = the fp16 bit
pattern of ~exp(logit); negatives saturate to 0 killing masked entries).
Remaining diagonal/chunk triangle masks are 0/1 multiplies on DVE.
O^T[65,q] (64 dims + denominator row via ones column in V) accumulates on PE
into [65,1024] PSUM tiles; normalization + final transpose happen on host.
"""

import sys

sys.path.insert(0, "/opt/trn_rl_repo")

import numpy as np

B, L, H, D = 4, 2048, 8, 64
TRAIN = 1536
TEST = L - TRAIN            # 512
NCH = 64                    # test chunks
CH = TEST // NCH            # 8
HPC = 4                     # heads per core
NCORES = 8
KT = L // 128               # 16 k-tiles

VW = KT * 65                # v columns per head
SPW = 512                   # PSUM S-tile width (1 bank)
PIPE = 4                    # sp tiles held back before AV

A_EXP = 1024.0 * np.log2(np.e) * 0.125     # PSUM = A_EXP * S
SQA = float(np.sqrt(A_EXP))                # folded into both Q and K
SC_SCALE = float(np.log(2.0) / 1024.0)     # ScalarE: exp(PSUM*SC_SCALE)
DVE_B = 15360.0 - 58.7                     # log-mean-centered Schraudolph bias
MASKVAL = -60000.0

LAST_RESULT = None
_PROG = None


def _split_multi_waits(nc, mybir):
    """This container's walrus accepts at most one semaphore wait per
    instruction; Tile's tail drains can carry several. Hoist extras onto
    NoOps inserted immediately before, on the same engine."""
    for f in nc.m.functions:
        for bb in f.blocks:
            insts = list(bb.instructions)
            out = []
            changed = False
            for inst in insts:
                si = inst.sync_info
                if si is not None and len(si.on_wait) > 1:
                    waits = list(si.on_wait)
                    for w in waits[:-1]:
                        nop = mybir.InstNoOp(
                            name=f"waitnop-{nc.next_id()}", ins=[], outs=[]
                        )
                        nop.engine = inst.engine
                        nop.sync_info = mybir.SyncInfo(on_wait=[w], on_update=[])
                        out.append(nop)
                    inst.sync_info = mybir.SyncInfo(
                        on_wait=[waits[-1]], on_update=list(si.on_update)
                    )
                    changed = True
                out.append(inst)
            if changed:
                bb.instructions = out


def _pieces(h_unused=None):
    """Per-head piece list in processing order.
    Returns [(kp, qs, w, mask, start, stop, half), ...] where qs is the
    global q start, mask in (None,'diag','chunk'), start/stop are the AV
    accumulation flags, half = qs // 1024 region of the av tile pair."""
    out = []
    for gq in range(4):
        if gq < 3:
            kps = list(range(4 * (gq + 1)))
        else:
            kps = list(range(16))
        for i, kp in enumerate(kps):
            if kp >= 12:
                qs = 128 * kp
                w = 128
                mask = "chunk"
            else:
                off = max(0, 128 * kp - 512 * gq)
                qs = 512 * gq + off
                w = 512 - off
                mask = "diag" if kp // 4 == gq else None
            out.append(
                dict(kp=kp, qs=qs, w=w, mask=mask, gq=gq,
                     start=(i == 0), stop=(i == len(kps) - 1))
            )
    return out


def _pack(pieces):
    """Pack pieces into [128, SPW] sp tiles; no piece crosses a 512-col
    PSUM bank. Returns list of tiles, each a list of (piece, pos)."""
    tiles = []
    cur = []
    pos = 0
    for p in pieces:
        w = p["w"]
        bank_rem = -pos % 512
        if 0 < bank_rem < w:
            pos += bank_rem
        if pos + w > SPW:
            tiles.append(cur)
            cur = []
            pos = 0
        cur.append((p, pos))
        pos += w
    if cur:
        tiles.append(cur)
    return tiles


def _build_program():
    import concourse.bass as bass
    import concourse.mybir as mybir
    import concourse.tile as tile

    f32 = mybir.dt.float32
    fp16 = mybir.dt.float16
    u16 = mybir.dt.uint16
    Exp = mybir.ActivationFunctionType.Exp

    nc = bass.Bass()

    qx_d = nc.dram_tensor("qx", [HPC, 128, L], fp16, kind="ExternalInput")
    kx_d = nc.dram_tensor("kx", [HPC, 128, L], fp16, kind="ExternalInput")
    vw_d = nc.dram_tensor("vw", [HPC, 128, VW], fp16, kind="ExternalInput")
    msk_d = nc.dram_tensor("msk", [128, 256], fp16, kind="ExternalInput")
    ot_d = nc.dram_tensor("ot", [HPC, 65, L], fp16, kind="ExternalOutput")

    with tile.TileContext(nc) as tc:
        with (
            tc.tile_pool(name="consts", bufs=1) as consts,
            tc.tile_pool(name="heads", bufs=4) as heads,
            tc.tile_pool(name="ptp", bufs=6) as ptp,
            tc.tile_pool(name="osbp", bufs=3) as osbp,
            tc.tile_pool(name="spp", bufs=4, space="PSUM") as spp,
            tc.tile_pool(name="avp", bufs=2, space="PSUM") as avp,
        ):
            # ---- PE clock warm-up + ACT table preload ---------------------
            warm_sb = consts.tile([128, 128], fp16, name="warm_sb")
            nc.gpsimd.memset(warm_sb, 0.0)
            warm_ps = spp.tile([128, SPW], f32, tag="sp", name="warm_ps")
            warm_pt = consts.tile([128, 128], fp16, name="warm_pt")
            nc.scalar.activation(warm_pt, warm_sb, Exp, scale=SC_SCALE)
            for _ in range(32):
                nc.tensor.matmul(
                    warm_ps[:, 0:128], lhsT=warm_sb, rhs=warm_sb,
                    start=True, stop=True, skip_group_check=True,
                )

            # ---- input DMAs ----------------------------------------------
            msk_sb = consts.tile([128, 256], fp16)
            qx_sbs, kx_sbs, vw_sbs = [], [], []
            for h in range(HPC):
                qx_sbs.append(heads.tile([128, L], fp16, tag="qx", name=f"qx{h}"))
                kx_sbs.append(heads.tile([128, L], fp16, tag="kx", name=f"kx{h}"))
                vw_sbs.append(heads.tile([128, VW], fp16, tag="vw", name=f"vw{h}"))

            nc.gpsimd.dma_start(out=kx_sbs[0][:, 0:512], in_=kx_d.ap()[0][:, 0:512])
            nc.gpsimd.dma_start(out=qx_sbs[0][:, 0:1024], in_=qx_d.ap()[0][:, 0:1024])
            nc.scalar.dma_start(out=msk_sb, in_=msk_d.ap())
            nc.gpsimd.dma_start(out=kx_sbs[0][:, 512:2048], in_=kx_d.ap()[0][:, 512:2048])
            nc.gpsimd.dma_start(out=qx_sbs[0][:, 1024:2048], in_=qx_d.ap()[0][:, 1024:2048])
            nc.gpsimd.dma_start(out=vw_sbs[0], in_=vw_d.ap()[0])
            for h in range(1, HPC):
                nc.gpsimd.dma_start(out=kx_sbs[h], in_=kx_d.ap()[h])
                nc.gpsimd.dma_start(out=qx_sbs[h], in_=qx_d.ap()[h])
                nc.gpsimd.dma_start(out=vw_sbs[h], in_=vw_d.ap()[h])

            mdiag = msk_sb[:, 0:128]
            mchunk = msk_sb[:, 128:256]

            # greedy engine balancing for converts / copies
            eng_t = {"sc": 0.0, "ve": 0.0}

            def conv_cost(eng, cols):
                if eng == "sc":
                    return (cols + 352) / 1.2
                return cols / 0.96 + 150.0

            pending = []
            osb_i = 0

            def convert(pt, sp, lo, hi, force_ve=False):
                """Emit the exp/PSUM-drain for sp[:, lo:hi] on the engine with
                the lower projected load. Halves containing masked pieces are
                pinned to DVE so the subsequent mask mul is ordered by the
                engine queue rather than a cross-engine semaphore."""
                cols = hi - lo
                if cols <= 0:
                    return
                if not force_ve and \
                   eng_t["sc"] + conv_cost("sc", cols) <= \
                   eng_t["ve"] + conv_cost("ve", cols):
                    eng_t["sc"] += conv_cost("sc", cols)
                    nc.scalar.activation(
                        pt.bitcast(mybir.dt.float16)[:, lo:hi],
                        sp[:, lo:hi], Exp, scale=SC_SCALE,
                    )
                else:
                    eng_t["ve"] += conv_cost("ve", cols)
                    nc.vector.tensor_scalar_add(pt[:, lo:hi], sp[:, lo:hi], DVE_B)

            def emit_avs(rec):
                nonlocal osb_i
                pt_, av_, h_, tlist = rec
                vw_sb = vw_sbs[h_]
                ptf = pt_.bitcast(mybir.dt.float16)
                for p, pos in tlist:
                    kp, qs, w = p["kp"], p["qs"], p["w"]
                    half = p["gq"] // 2
                    av = av_[half]
                    nc.tensor.matmul(
                        av[:, qs - 1024 * half: qs - 1024 * half + w],
                        lhsT=vw_sb[:, 65 * kp: 65 * kp + 65],
                        rhs=ptf[:, pos: pos + w],
                        start=p["start"], stop=p["stop"],
                        skip_group_check=True,
                    )
                    if p["stop"] and qs + w == 1024 * (half + 1):
                        osb = osbp.tile([65, 1024], mybir.dt.float16,
                                        name=f"osb{osb_i}")
                        osb_i += 1
                        if eng_t["sc"] + conv_cost("sc", 1024) <= \
                           eng_t["ve"] + conv_cost("ve", 1024):
                            eng_t["sc"] += conv_cost("sc", 1024)
                            nc.scalar.copy(osb, av[:, :])
                        else:
                            eng_t["ve"] += conv_cost("ve", 1024)
                            nc.vector.tensor_copy(osb, av[:, :])
                        nc.gpsimd.dma_start(
                            out=ot_d.ap()[h_][:, 1024 * half: 1024 * (half + 1)],
                            in_=osb,
                        )

            for h in range(HPC):
                qx, kx = qx_sbs[h], kx_sbs[h]
                avs = {}
                for half in range(2):
                    avs[half] = avp.tile([65, 1024], f32, tag="av",
                                         name=f"av{h}_{half}")
                for tlist in _pack(_pieces()):
                    sp = spp.tile([128, SPW], f32, tag="sp")
                    used = tlist[-1][1] + tlist[-1][0]["w"]
                    pt = ptp.tile([128, SPW], u16, tag="pt")
                    for p, pos in tlist:
                        kp, qs, w = p["kp"], p["qs"], p["w"]
                        nc.tensor.matmul(
                            sp[:, pos: pos + w],
                            lhsT=kx[:, 128 * kp: 128 * kp + 128],
                            rhs=qx[:, qs: qs + w],
                            start=True, stop=True, skip_group_check=True,
                        )
                    has_mask = any(p["mask"] for p, pos in tlist)
                    convert(pt, sp, 0, used, force_ve=has_mask)
                    ptf = pt.bitcast(mybir.dt.float16)
                    for p, pos in tlist:
                        if p["mask"] == "diag":
                            nc.vector.tensor_mul(
                                ptf[:, pos: pos + 128],
                                ptf[:, pos: pos + 128], mdiag,
                            )
                        elif p["mask"] == "chunk":
                            nc.vector.tensor_mul(
                                ptf[:, pos: pos + 128],
                                ptf[:, pos: pos + 128], mchunk,
                            )
                    pending.append((pt, avs, h, tlist))
                    while len(pending) > PIPE:
                        emit_avs(pending.pop(0))
            while pending:
                emit_avs(pending.pop(0))

    import concourse.mybir as mybir_mod

    _split_multi_waits(nc, mybir_mod)
    return nc


def _host_inputs(queries, keys, values, attach):
    f16 = np.float16
    p = np.arange(128)
    f = np.arange(128)
    mdiag = (f[None, :] >= p[:, None]).astype(np.float32)
    # test chunk mask: same CH-chunk and causal within the 128-block
    mchunk = ((p[:, None] // CH == f[None, :] // CH) &
              (p[:, None] <= f[None, :])).astype(np.float32)
    msk = np.concatenate([mdiag, mchunk], axis=1)  # [128, 256]

    kg = np.arange(TRAIN)
    qn = np.arange(NCH)
    in_maps = []
    for c in range(NCORES):
        b, g = divmod(c, 2)
        hs = slice(HPC * g, HPC * (g + 1))
        q = queries[b][:, hs, :]          # [L, 4, D]
        k = keys[b][:, hs, :]
        v = values[b][:, hs, :]
        qt = np.ascontiguousarray(q.transpose(1, 2, 0)) * SQA  # [4, 64, L]
        kt = np.ascontiguousarray(k.transpose(1, 2, 0)) * SQA
        vw = np.empty((HPC, L, 65), np.float32)
        vw[:, :, :64] = v.transpose(1, 0, 2)
        vw[:, :, 64] = 1.0
        vw = np.ascontiguousarray(
            vw.reshape(HPC, KT, 128, 65).transpose(0, 2, 1, 3)
            .reshape(HPC, 128, KT * 65)
        )
        # mask rows: K side = MASKVAL*[k > att_n] on train keys,
        # Q side = [chunk(q) == n] on test queries
        att = attach[b]                                   # [64]
        krow = np.where(kg[None, :] > att[:, None], MASKVAL, 0.0)  # [64,1536]
        qrow = np.zeros((NCH, L), np.float32)
        tq = np.arange(TEST)
        qrow[:, TRAIN:] = (tq[None, :] // CH == qn[:, None]).astype(np.float32)

        qx = np.zeros((HPC, 128, L), np.float32)
        kx = np.zeros((HPC, 128, L), np.float32)
        qx[:, :64, :] = qt
        qx[:, 64:, :] = qrow[None]
        kx[:, :64, :] = kt
        kx[:, 64:, :TRAIN] = krow[None]
        in_maps.append(
            {
                "qx": qx.astype(f16),
                "kx": kx.astype(f16),
                "vw": vw.astype(f16),
                "msk": msk.astype(f16),
            }
        )
    return in_maps


def kernel(queries, keys, values, attach_test_after, train_len):
    global LAST_RESULT, _PROG
    import os

    queries = np.asarray(queries, dtype=np.float32)
    keys = np.asarray(keys, dtype=np.float32)
    values = np.asarray(values, dtype=np.float32)
    attach = np.asarray(attach_test_after).astype(np.int64)
    tl = int(np.asarray(train_len))
    assert queries.shape == (B, L, H, D), queries.shape
    assert tl == TRAIN and attach.shape == (B, NCH)

    from concourse.bass_utils import run_bass_kernel_spmd

    if _PROG is None:
        _PROG = _build_program()

    in_maps = _host_inputs(queries, keys, values, attach)
    trace = bool(int(os.environ.get("KERNEL_TRACE", "0")))
    res = run_bass_kernel_spmd(
        _PROG, in_maps, core_ids=list(range(NCORES)), trace=trace
    )
    LAST_RESULT = res

    out = np.empty((B, L, H * D), np.float32)
    for c in range(NCORES):
        b, g = divmod(c, 2)
        ot = res.results[c]["ot"].astype(np.float32)  # [4, 65, L]
        o = ot[:, :64, :] / ot[:, 64:65, :]           # [4, 64, L]
        out[b, :, 256 * g: 256 * (g + 1)] = (
            o.transpose(2, 0, 1).reshape(L, HPC * D)
        )
    return out


# revision 3
# speedup vs baseline: 1.0694x; 1.0264x over previous
"""Continual-attention Trainium2 kernel v3 (8 NeuronCores, SPMD).

Sharding: core c -> batch b = c//2, head-group g = c%2 (4 heads each).

Per (b,h): S^T[k,q] = K Q^T on PE with 64 data contraction rows (scaled by
sqrt(A), A = 1024*log2(e)/8) plus 64 mask rows folded into the same matmul:
Q rows 64-127 hold per-chunk indicators for test queries, K rows 64-127 hold
-60000*[k > attach[b,n]] steps, so masked logits go hugely negative in PSUM.
The exp+PSUM-drain pass is split between ScalarE (true exp via activation,
scale=ln2/1024) and DVE (Schraudolph: round(A*S + B) as uint16 = the fp16 bit
pattern of ~exp(logit); negatives saturate to 0 killing masked entries).
Remaining diagonal/chunk triangle masks are 0/1 multiplies on DVE.
O^T[65,q] (64 dims + denominator row via ones column in V) accumulates on PE
into [65,1024] PSUM tiles; normalization + final transpose happen on host.
"""

import sys

sys.path.insert(0, "/opt/trn_rl_repo")

import numpy as np

B, L, H, D = 4, 2048, 8, 64
TRAIN = 1536
TEST = L - TRAIN            # 512
NCH = 64                    # test chunks
CH = TEST // NCH            # 8
HPC = 4                     # heads per core
NCORES = 8
KT = L // 128               # 16 k-tiles

VW = KT * 65                # v columns per head
SPW = 512                   # PSUM S-tile width (1 bank)
PIPE = 6                    # sp tiles held back before AV

A_EXP = 1024.0 * np.log2(np.e) * 0.125     # PSUM = A_EXP * S
SQA = float(np.sqrt(A_EXP))                # folded into both Q and K
SC_SCALE = float(np.log(2.0) / 1024.0)     # ScalarE: exp(PSUM*SC_SCALE)
DVE_B = 15360.0 - 58.7                     # log-mean-centered Schraudolph bias
MASKVAL = -60000.0

LAST_RESULT = None
_PROG = None


def _split_multi_waits(nc, mybir):
    """This container's walrus accepts at most one semaphore wait per
    instruction; Tile's tail drains can carry several. Hoist extras onto
    NoOps inserted immediately before, on the same engine."""
    for f in nc.m.functions:
        for bb in f.blocks:
            insts = list(bb.instructions)
            out = []
            changed = False
            for inst in insts:
                si = inst.sync_info
                if si is not None and len(si.on_wait) > 1:
                    waits = list(si.on_wait)
                    for w in waits[:-1]:
                        nop = mybir.InstNoOp(
                            name=f"waitnop-{nc.next_id()}", ins=[], outs=[]
                        )
                        nop.engine = inst.engine
                        nop.sync_info = mybir.SyncInfo(on_wait=[w], on_update=[])
                        out.append(nop)
                    inst.sync_info = mybir.SyncInfo(
                        on_wait=[waits[-1]], on_update=list(si.on_update)
                    )
                    changed = True
                out.append(inst)
            if changed:
                bb.instructions = out


def _pieces(h_unused=None):
    """Per-head piece list in processing order.
    Returns [(kp, qs, w, mask, start, stop, half), ...] where qs is the
    global q start, mask in (None,'diag','chunk'), start/stop are the AV
    accumulation flags, half = qs // 1024 region of the av tile pair."""
    out = []
    for gq in range(4):
        if gq < 3:
            kps = list(range(4 * (gq + 1)))
        else:
            kps = list(range(16))
        for i, kp in enumerate(kps):
            if kp >= 12:
                qs = 128 * kp
                w = 128
                mask = "chunk"
            else:
                off = max(0, 128 * kp - 512 * gq)
                qs = 512 * gq + off
                w = 512 - off
                mask = "diag" if kp // 4 == gq else None
            out.append(
                dict(kp=kp, qs=qs, w=w, mask=mask, gq=gq,
                     start=(i == 0), stop=(i == len(kps) - 1))
            )
    return out


def _pack(pieces):
    """Pack pieces into [128, SPW] sp tiles; no piece crosses a 512-col
    PSUM bank. Returns list of tiles, each a list of (piece, pos)."""
    tiles = []
    cur = []
    pos = 0
    for p in pieces:
        w = p["w"]
        bank_rem = -pos % 512
        if 0 < bank_rem < w:
            pos += bank_rem
        if pos + w > SPW:
            tiles.append(cur)
            cur = []
            pos = 0
        cur.append((p, pos))
        pos += w
    if cur:
        tiles.append(cur)
    return tiles


def _build_program():
    import concourse.bass as bass
    import concourse.mybir as mybir
    import concourse.tile as tile

    f32 = mybir.dt.float32
    fp16 = mybir.dt.float16
    u16 = mybir.dt.uint16
    Exp = mybir.ActivationFunctionType.Exp

    nc = bass.Bass()

    qx_d = nc.dram_tensor("qx", [HPC, 128, L], fp16, kind="ExternalInput")
    kx_d = nc.dram_tensor("kx", [HPC, 128, L], fp16, kind="ExternalInput")
    vw_d = nc.dram_tensor("vw", [HPC, 128, VW], fp16, kind="ExternalInput")
    msk_d = nc.dram_tensor("msk", [128, 256], fp16, kind="ExternalInput")
    ot_d = nc.dram_tensor("ot", [HPC, 65, L], fp16, kind="ExternalOutput")

    with tile.TileContext(nc) as tc:
        with (
            tc.tile_pool(name="consts", bufs=1) as consts,
            tc.tile_pool(name="heads", bufs=4) as heads,
            tc.tile_pool(name="ptp", bufs=8) as ptp,
            tc.tile_pool(name="osbp", bufs=3) as osbp,
            tc.tile_pool(name="spp", bufs=4, space="PSUM") as spp,
            tc.tile_pool(name="avp", bufs=2, space="PSUM") as avp,
        ):
            # ---- PE clock warm-up + ACT table preload ---------------------
            warm_sb = consts.tile([128, 128], fp16, name="warm_sb")
            nc.gpsimd.memset(warm_sb, 0.0)
            warm_ps = spp.tile([128, SPW], f32, tag="sp", name="warm_ps")
            warm_pt = consts.tile([128, 128], fp16, name="warm_pt")
            nc.scalar.activation(warm_pt, warm_sb, Exp, scale=SC_SCALE)
            for _ in range(32):
                nc.tensor.matmul(
                    warm_ps[:, 0:128], lhsT=warm_sb, rhs=warm_sb,
                    start=True, stop=True, skip_group_check=True,
                )

            # ---- input DMAs ----------------------------------------------
            msk_sb = consts.tile([128, 256], fp16)
            qx_sbs, kx_sbs, vw_sbs = [], [], []
            for h in range(HPC):
                qx_sbs.append(heads.tile([128, L], fp16, tag="qx", name=f"qx{h}"))
                kx_sbs.append(heads.tile([128, L], fp16, tag="kx", name=f"kx{h}"))
                vw_sbs.append(heads.tile([128, VW], fp16, tag="vw", name=f"vw{h}"))

            nc.gpsimd.dma_start(out=kx_sbs[0][:, 0:512], in_=kx_d.ap()[0][:, 0:512])
            nc.gpsimd.dma_start(out=qx_sbs[0][:, 0:1024], in_=qx_d.ap()[0][:, 0:1024])
            nc.scalar.dma_start(out=msk_sb, in_=msk_d.ap())
            nc.gpsimd.dma_start(out=kx_sbs[0][:, 512:2048], in_=kx_d.ap()[0][:, 512:2048])
            nc.gpsimd.dma_start(out=qx_sbs[0][:, 1024:2048], in_=qx_d.ap()[0][:, 1024:2048])
            nc.gpsimd.dma_start(out=vw_sbs[0], in_=vw_d.ap()[0])
            for h in range(1, HPC):
                nc.gpsimd.dma_start(out=kx_sbs[h], in_=kx_d.ap()[h])
                nc.gpsimd.dma_start(out=qx_sbs[h], in_=qx_d.ap()[h])
                nc.gpsimd.dma_start(out=vw_sbs[h], in_=vw_d.ap()[h])

            mdiag = msk_sb[:, 0:128]
            mchunk = msk_sb[:, 128:256]

            # greedy engine balancing for converts / copies
            eng_t = {"sc": 0.0, "ve": 0.0}

            def conv_cost(eng, cols):
                if eng == "sc":
                    return (cols + 352) / 1.2
                return cols / 0.96 + 150.0

            pending = []
            osb_i = 0

            def convert(pt, sp, lo, hi, force_ve=False):
                """Emit the exp/PSUM-drain for sp[:, lo:hi] on the engine with
                the lower projected load. Halves containing masked pieces are
                pinned to DVE so the subsequent mask mul is ordered by the
                engine queue rather than a cross-engine semaphore."""
                cols = hi - lo
                if cols <= 0:
                    return
                if not force_ve and \
                   eng_t["sc"] + conv_cost("sc", cols) <= \
                   eng_t["ve"] + conv_cost("ve", cols):
                    eng_t["sc"] += conv_cost("sc", cols)
                    nc.scalar.activation(
                        pt.bitcast(mybir.dt.float16)[:, lo:hi],
                        sp[:, lo:hi], Exp, scale=SC_SCALE,
                    )
                else:
                    eng_t["ve"] += conv_cost("ve", cols)
                    nc.vector.tensor_scalar_add(pt[:, lo:hi], sp[:, lo:hi], DVE_B)

            def emit_avs(rec):
                nonlocal osb_i
                pt_, av_, h_, tlist = rec
                vw_sb = vw_sbs[h_]
                ptf = pt_.bitcast(mybir.dt.float16)
                for p, pos in tlist:
                    kp, qs, w = p["kp"], p["qs"], p["w"]
                    half = p["gq"] // 2
                    av = av_[half]
                    nc.tensor.matmul(
                        av[:, qs - 1024 * half: qs - 1024 * half + w],
                        lhsT=vw_sb[:, 65 * kp: 65 * kp + 65],
                        rhs=ptf[:, pos: pos + w],
                        start=p["start"], stop=p["stop"],
                        skip_group_check=True,
                    )
                    if p["stop"] and qs + w == 1024 * (half + 1):
                        osb = osbp.tile([65, 1024], mybir.dt.float16,
                                        name=f"osb{osb_i}")
                        osb_i += 1
                        if eng_t["sc"] + conv_cost("sc", 1024) <= \
                           eng_t["ve"] + conv_cost("ve", 1024):
                            eng_t["sc"] += conv_cost("sc", 1024)
                            nc.scalar.copy(osb, av[:, :])
                        else:
                            eng_t["ve"] += conv_cost("ve", 1024)
                            nc.vector.tensor_copy(osb, av[:, :])
                        nc.gpsimd.dma_start(
                            out=ot_d.ap()[h_][:, 1024 * half: 1024 * (half + 1)],
                            in_=osb,
                        )

            for h in range(HPC):
                qx, kx = qx_sbs[h], kx_sbs[h]
                avs = {}
                for half in range(2):
                    avs[half] = avp.tile([65, 1024], f32, tag="av",
                                         name=f"av{h}_{half}")
                for tlist in _pack(_pieces()):
                    sp = spp.tile([128, SPW], f32, tag="sp")
                    used = tlist[-1][1] + tlist[-1][0]["w"]
                    pt = ptp.tile([128, SPW], u16, tag="pt")
                    for p, pos in tlist:
                        kp, qs, w = p["kp"], p["qs"], p["w"]
                        nc.tensor.matmul(
                            sp[:, pos: pos + w],
                            lhsT=kx[:, 128 * kp: 128 * kp + 128],
                            rhs=qx[:, qs: qs + w],
                            start=True, stop=True, skip_group_check=True,
                        )
                    has_mask = any(p["mask"] for p, pos in tlist)
                    convert(pt, sp, 0, used, force_ve=has_mask)
                    ptf = pt.bitcast(mybir.dt.float16)
                    for p, pos in tlist:
                        if p["mask"] == "diag":
                            nc.vector.tensor_mul(
                                ptf[:, pos: pos + 128],
                                ptf[:, pos: pos + 128], mdiag,
                            )
                        elif p["mask"] == "chunk":
                            nc.vector.tensor_mul(
                                ptf[:, pos: pos + 128],
                                ptf[:, pos: pos + 128], mchunk,
                            )
                    pending.append((pt, avs, h, tlist))
                    while len(pending) > PIPE:
                        emit_avs(pending.pop(0))
            while pending:
                emit_avs(pending.pop(0))

    import concourse.mybir as mybir_mod

    _split_multi_waits(nc, mybir_mod)
    return nc


def _host_inputs(queries, keys, values, attach):
    f16 = np.float16
    p = np.arange(128)
    f = np.arange(128)
    mdiag = (f[None, :] >= p[:, None]).astype(np.float32)
    # test chunk mask: same CH-chunk and causal within the 128-block
    mchunk = ((p[:, None] // CH == f[None, :] // CH) &
              (p[:, None] <= f[None, :])).astype(np.float32)
    msk = np.concatenate([mdiag, mchunk], axis=1)  # [128, 256]

    kg = np.arange(TRAIN)
    qn = np.arange(NCH)
    in_maps = []
    for c in range(NCORES):
        b, g = divmod(c, 2)
        hs = slice(HPC * g, HPC * (g + 1))
        q = queries[b][:, hs, :]          # [L, 4, D]
        k = keys[b][:, hs, :]
        v = values[b][:, hs, :]
        qt = np.ascontiguousarray(q.transpose(1, 2, 0)) * SQA  # [4, 64, L]
        kt = np.ascontiguousarray(k.transpose(1, 2, 0)) * SQA
        vw = np.empty((HPC, L, 65), np.float32)
        vw[:, :, :64] = v.transpose(1, 0, 2)
        vw[:, :, 64] = 1.0
        vw = np.ascontiguousarray(
            vw.reshape(HPC, KT, 128, 65).transpose(0, 2, 1, 3)
            .reshape(HPC, 128, KT * 65)
        )
        # mask rows: K side = MASKVAL*[k > att_n] on train keys,
        # Q side = [chunk(q) == n] on test queries
        att = attach[b]                                   # [64]
        krow = np.where(kg[None, :] > att[:, None], MASKVAL, 0.0)  # [64,1536]
        qrow = np.zeros((NCH, L), np.float32)
        tq = np.arange(TEST)
        qrow[:, TRAIN:] = (tq[None, :] // CH == qn[:, None]).astype(np.float32)

        qx = np.zeros((HPC, 128, L), np.float32)
        kx = np.zeros((HPC, 128, L), np.float32)
        qx[:, :64, :] = qt
        qx[:, 64:, :] = qrow[None]
        kx[:, :64, :] = kt
        kx[:, 64:, :TRAIN] = krow[None]
        in_maps.append(
            {
                "qx": qx.astype(f16),
                "kx": kx.astype(f16),
                "vw": vw.astype(f16),
                "msk": msk.astype(f16),
            }
        )
    return in_maps


def kernel(queries, keys, values, attach_test_after, train_len):
    global LAST_RESULT, _PROG
    import os

    queries = np.asarray(queries, dtype=np.float32)
    keys = np.asarray(keys, dtype=np.float32)
    values = np.asarray(values, dtype=np.float32)
    attach = np.asarray(attach_test_after).astype(np.int64)
    tl = int(np.asarray(train_len))
    assert queries.shape == (B, L, H, D), queries.shape
    assert tl == TRAIN and attach.shape == (B, NCH)

    from concourse.bass_utils import run_bass_kernel_spmd

    if _PROG is None:
        _PROG = _build_program()

    in_maps = _host_inputs(queries, keys, values, attach)
    trace = bool(int(os.environ.get("KERNEL_TRACE", "0")))
    res = run_bass_kernel_spmd(
        _PROG, in_maps, core_ids=list(range(NCORES)), trace=trace
    )
    LAST_RESULT = res

    out = np.empty((B, L, H * D), np.float32)
    for c in range(NCORES):
        b, g = divmod(c, 2)
        ot = res.results[c]["ot"].astype(np.float32)  # [4, 65, L]
        o = ot[:, :64, :] / ot[:, 64:65, :]           # [4, 64, L]
        out[b, :, 256 * g: 256 * (g + 1)] = (
            o.transpose(2, 0, 1).reshape(L, HPC * D)
        )
    return out
